# revision 14
# baseline (speedup 1.0000x reference)
"""Trainium2 Bass kernel for BayesianLinear sampling (B=2048, OUT=IN=256).

out[b,o] = sum_i (mu[o,i] + std[o,i]*eps_w[b,o,i]) * x[b,i]
         + bias_mu[o] + bias_std[o]*eps_b[b,o]

Data-parallel over batch across 8 NeuronCores (256 samples each).

Per-core pipeline (out is produced transposed as (o, b); the host
transposes back):

  eps load   -- split across three DMA queues so the transfers overlap:
               even 16-sample chunks stream through the gpsimd (SWDGE)
               queue with an in-flight f32->f16 cast; odd chunks come in
               as raw f32 via the SP and ACT HWDGE queues (one o-half
               each) and are cast to f16 by the Scalar engine.
  PE         -- per 128x128 block, transpose eps (o,i)->(i,o) into an
               f16 PSUM slab (is_transpose matmul against an f16
               identity, 1 cycle/row).
  DVE        -- one tensor_tensor per 4-sample group: Q = epsT * stdT
               (PSUM f16 -> SBUF f16, 2x perf mode).  This doubles as
               the PSUM->SBUF copy.
  PE         -- per (sample, i-half, o-half): n=1 matmul with the Q
               block as the stationary operand and the sample's x
               column as the moving operand, accumulating
               outT[o, b] = sum_i Q[i,o] x[b,i] into PSUM.  The
               mu @ x^T base term is 4 more matmuls into a second PSUM
               tile.
  epilogue   -- DVE adds base + (bias_mu + bias_std*eps_b)^T (host
               precomputed) and the result is DMA'd out as (o, b).

The small parameters (mu, std=exp(0.5*logvar), x, biases) are packed on
the host; eps_w is shipped to the device untouched.
"""

import sys

sys.path.insert(0, "/opt/trn_rl_repo")

import numpy as np

import concourse.bass as bass
import concourse.bacc as bacc
import concourse.mybir as mybir
from concourse import tile
from concourse.bass_utils import run_bass_kernel_spmd

N_CORES = 8
B, OUT, IN = 2048, 256, 256
B_CORE = B // N_CORES          # 256 samples per core
SD = 16                        # samples per DMA chunk
S = 4                          # samples per transpose/fold group
NCHUNK = B_CORE // SD          # 16 chunks
F32 = mybir.dt.float32
F16 = mybir.dt.float16
MULT = mybir.AluOpType.mult
ADD = mybir.AluOpType.add

SREP_W = S * 2 * 2 * 128       # 2048: (h_i, s, h_o) blocks of 128


def _blk(h_i, s, h_o):
    """Free-dim offset of block (h_i, s, h_o) in a Q / QT slab."""
    return ((h_i * S + s) * 2 + h_o) * 128


def _build_nc():
    nc = bacc.Bacc(trn_type="TRN2")

    eps = nc.declare_dram_parameter("eps", [B_CORE, OUT, IN], F32, isOutput=False)
    srepT = nc.declare_dram_parameter("srepT", [128, SREP_W], F16, isOutput=False)
    srepOI = nc.declare_dram_parameter("srepOI", [128, SREP_W], F16, isOutput=False)
    xT = nc.declare_dram_parameter("xT", [128, 2 * B_CORE], F16, isOutput=False)
    muT = nc.declare_dram_parameter("muT", [128, 2 * OUT], F16, isOutput=False)
    biasesT = nc.declare_dram_parameter("biasesT", [128, 2 * B_CORE], F32,
                                        isOutput=False)
    ident = nc.declare_dram_parameter("ident", [128, 128], F16, isOutput=False)
    out = nc.declare_dram_parameter("out", [OUT, B_CORE], F32, isOutput=True)

    eps3 = eps.rearrange("b o i -> o b i")  # partition = o view for DMA

    with tile.TileContext(nc) as tc:
        with tc.tile_pool(name="const", bufs=1) as cpool:
            srepT_sb = cpool.tile([128, SREP_W], F16, tag="srepT", name="srepTsb")
            srepOI_sb = cpool.tile([128, SREP_W], F16, tag="srepOI",
                                   name="srepOIsb")
            xT_sb = cpool.tile([128, 2 * B_CORE], F16, tag="xT", name="xTsb")
            muT_sb = cpool.tile([128, 2 * OUT], F16, tag="muT", name="muTsb")
            bias_sb = cpool.tile([128, 2 * B_CORE], F32, tag="biasesT",
                                 name="biassb")
            id_sb = cpool.tile([128, 128], F16, tag="ident", name="idsb")
            out_sb = [cpool.tile([128, B_CORE], F32, tag=f"osb{h}",
                                 name=f"osb{h}") for h in range(2)]
            tmp_sb = [cpool.tile([128, B_CORE], F32, tag=f"tsb{h}",
                                 name=f"tsb{h}") for h in range(2)]

            nc.sync.dma_start(out=srepT_sb[:], in_=srepT[:])
            nc.sync.dma_start(out=srepOI_sb[:], in_=srepOI[:])
            nc.sync.dma_start(out=xT_sb[:], in_=xT[:])
            nc.sync.dma_start(out=muT_sb[:], in_=muT[:])
            nc.sync.dma_start(out=bias_sb[:], in_=biasesT[:])
            nc.sync.dma_start(out=id_sb[:], in_=ident[:])

            with (
                tc.tile_pool(name="ef", bufs=4) as efpool,
                tc.tile_pool(name="e32", bufs=2) as e32pool,
                tc.tile_pool(name="qoi", bufs=2) as qoipool,
                tc.tile_pool(name="qt", bufs=2, space="PSUM") as qtpool,
                tc.tile_pool(name="q", bufs=3) as qpool,
                tc.tile_pool(name="acc", bufs=1, space="PSUM") as accpool,
            ):
                acc = [accpool.tile([128, B_CORE], F32, tag=f"acc{h}",
                                    name=f"acc{h}") for h in range(2)]
                pbase = [accpool.tile([128, B_CORE], F32, tag=f"pb{h}",
                                      name=f"pb{h}") for h in range(2)]

                # base term: pbase[h_o][o_l, b] = sum_i mu[o,i] x[b,i]
                for h_o in range(2):
                    for h_i in range(2):
                        nc.tensor.matmul(
                            pbase[h_o][:],
                            muT_sb[:, h_i * OUT + h_o * 128:
                                   h_i * OUT + (h_o + 1) * 128],
                            xT_sb[:, h_i * B_CORE:(h_i + 1) * B_CORE],
                            start=(h_i == 0), stop=(h_i == 1),
                        )

                # pool-path: even chunks plus the last one (short tail);
                # hwdge-path: odd chunks 1..13.  Front-load pool chunks in
                # program order so DVE has work while the first hwdge
                # chunk's longer DMA->cast chain fills.
                order = [0, 2, 1, 4, 3, 6, 5, 8, 7, 10, 9, 12, 11, 14, 13, 15]
                pool_chunks = [c for c in order if c % 2 == 0 or c == NCHUNK - 1]
                hw_chunks = [c for c in order if c not in pool_chunks]
                # fold-offload: on these (chunk, group) pairs the fold runs
                # on GPSIMD in (o,i) layout before the transpose, and ACT
                # does the PSUM->SBUF copy, relieving DVE.
                offload = set()
                for pi, c in enumerate(pool_chunks):
                    offload.add((c, 2))
                    if pi % 3 == 2:
                        offload.add((c, 0))
                for c in order:
                    ef = efpool.tile([128, 2 * SD * IN], F16, tag="ef",
                                     name="ef")
                    if c in pool_chunks:
                        # SWDGE queue: in-flight f32 -> f16 cast
                        for h_o in range(2):
                            nc.gpsimd.dma_start(
                                out=ef[:, h_o * SD * IN:(h_o + 1) * SD * IN]
                                .rearrange("p (s i) -> p s i", i=IN),
                                in_=eps3[h_o * 128:(h_o + 1) * 128,
                                         c * SD:(c + 1) * SD, :],
                            )
                    else:
                        # HWDGE queues (SP + ACT): raw f32, then cast.
                        # Cast engine ratio ACT:Pool ~ 11:3 across chunks.
                        ci = hw_chunks.index(c)
                        e32 = e32pool.tile([128, 2 * SD * IN], F32, tag="e32",
                                           name="e32")
                        for h_o, eng in ((0, nc.sync), (1, nc.scalar)):
                            eng.dma_start(
                                out=e32[:, h_o * SD * IN:(h_o + 1) * SD * IN]
                                .rearrange("p (s i) -> p s i", i=IN),
                                in_=eps3[h_o * 128:(h_o + 1) * 128,
                                         c * SD:(c + 1) * SD, :],
                            )
                        nc.scalar.copy(
                            out=ef[:, 0:SD * IN],
                            in_=e32[:, 0:SD * IN],
                        )
                        if ci % 2 == 1:
                            nc.gpsimd.tensor_copy(
                                ef[:, SD * IN:2 * SD * IN],
                                e32[:, SD * IN:2 * SD * IN],
                            )
                        else:
                            nc.scalar.copy(
                                out=ef[:, SD * IN:2 * SD * IN],
                                in_=e32[:, SD * IN:2 * SD * IN],
                            )

                    for gs in range(SD // S):
                        off = (c, gs) in offload
                        if off:
                            # GPSIMD folds in (o,i) layout; transposes then
                            # read the folded tile
                            qoi = qoipool.tile([128, 2 * S * IN], F16,
                                               tag="qoi", name="qoi")
                            nc.gpsimd.tensor_tensor(
                                qoi[:].rearrange("p (h s i) -> p h s i",
                                                 h=2, s=S, i=IN),
                                ef[:].rearrange("p (h s i) -> p h s i",
                                                h=2, s=SD, i=IN)
                                [:, :, gs * S:(gs + 1) * S, :],
                                srepOI_sb[:].rearrange(
                                    "p (h s i) -> p h s i", h=2, s=S, i=IN),
                                op=MULT,
                            )
                            src = qoi
                        qt = qtpool.tile([128, SREP_W], F16, tag="qt",
                                         name="qt")
                        for h_o in range(2):
                            for s in range(S):
                                for h_i in range(2):
                                    if off:
                                        tin = src[:, (h_o * S + s) * IN
                                                  + h_i * 128:
                                                  (h_o * S + s) * IN
                                                  + h_i * 128 + 128]
                                    else:
                                        tin = ef[:, (h_o * SD + gs * S + s) * IN
                                                 + h_i * 128:
                                                 (h_o * SD + gs * S + s) * IN
                                                 + h_i * 128 + 128]
                                    nc.tensor.transpose(
                                        qt[:, _blk(h_i, s, h_o):
                                           _blk(h_i, s, h_o) + 128],
                                        tin,
                                        id_sb[:],
                                    )
                        q = qpool.tile([128, SREP_W], F16, tag="q", name="q")
                        if off:
                            nc.scalar.copy(out=q[:], in_=qt[:])
                        else:
                            nc.vector.tensor_tensor(out=q[:], in0=qt[:],
                                                    in1=srepT_sb[:], op=MULT)
                        for s in range(S):
                            b = c * SD + gs * S + s
                            for h_o in range(2):
                                for h_i in range(2):
                                    nc.tensor.matmul(
                                        acc[h_o][:, b:b + 1],
                                        q[:, _blk(h_i, s, h_o):
                                          _blk(h_i, s, h_o) + 128],
                                        xT_sb[:, h_i * B_CORE + b:
                                              h_i * B_CORE + b + 1],
                                        start=(h_i == 0), stop=(h_i == 1),
                                    )

                # epilogue: out_sb = acc + biasesT + pbase  (each DVE op
                # touches at most one PSUM operand)
                for h_o in range(2):
                    nc.vector.tensor_tensor(
                        out=tmp_sb[h_o][:], in0=acc[h_o][:],
                        in1=bias_sb[:, h_o * B_CORE:(h_o + 1) * B_CORE],
                        op=ADD)
                    nc.vector.tensor_tensor(
                        out=out_sb[h_o][:], in0=pbase[h_o][:],
                        in1=tmp_sb[h_o][:], op=ADD)
                    nc.sync.dma_start(
                        out=out[h_o * 128:(h_o + 1) * 128, :],
                        in_=out_sb[h_o][:])

    nc.compile()
    return nc


_NC_CACHE = None


def _get_nc():
    global _NC_CACHE
    if _NC_CACHE is None:
        _NC_CACHE = _build_nc()
    return _NC_CACHE


def _prep_inputs(x, weight_mu, weight_logvar, bias_mu, bias_logvar, eps_w, eps_b):
    """Host-side prep: shard eps over batch, pack the small params."""
    x = np.asarray(x, np.float32)
    weight_mu = np.asarray(weight_mu, np.float32)
    weight_logvar = np.asarray(weight_logvar, np.float32)
    bias_mu = np.asarray(bias_mu, np.float32)
    bias_logvar = np.asarray(bias_logvar, np.float32)
    eps_w = np.asarray(eps_w, np.float32)
    eps_b = np.asarray(eps_b, np.float32)

    std = np.exp(0.5 * weight_logvar)                  # (OUT, IN)
    bstd = np.exp(0.5 * bias_logvar)                   # (OUT,)
    stdT = np.ascontiguousarray(std.T).astype(np.float16)   # (IN, OUT)
    muT16 = np.ascontiguousarray(weight_mu.T).astype(np.float16)

    # srepT[p, blk(h_i, s, h_o) + c] = stdT[h_i*128+p, h_o*128+c]
    srepT = np.zeros((128, SREP_W), np.float16)
    for h_i in range(2):
        for s in range(S):
            for h_o in range(2):
                off = _blk(h_i, s, h_o)
                srepT[:, off:off + 128] = stdT[h_i * 128:(h_i + 1) * 128,
                                               h_o * 128:(h_o + 1) * 128]

    # srepOI[p, (h_o, s, i)] = std[h_o*128+p, i]  ((o,i) layout, s-replicated)
    srepOI = np.zeros((128, SREP_W), np.float16)
    std16 = std.astype(np.float16)
    for h_o in range(2):
        for s in range(S):
            off = (h_o * S + s) * IN
            srepOI[:, off:off + IN] = std16[h_o * 128:(h_o + 1) * 128, :]

    # muT packed as [p(i_l), h_i*OUT + o]
    muT = np.zeros((128, 2 * OUT), np.float16)
    for h_i in range(2):
        muT[:, h_i * OUT:(h_i + 1) * OUT] = muT16[h_i * 128:(h_i + 1) * 128, :]

    ident = np.eye(128, dtype=np.float16)

    in_maps = []
    for cix in range(N_CORES):
        sl = slice(cix * B_CORE, (cix + 1) * B_CORE)
        x_c = x[sl]                                     # (B_CORE, IN)
        xTc = np.ascontiguousarray(x_c.T).astype(np.float16)  # (IN, B_CORE)
        xT = np.zeros((128, 2 * B_CORE), np.float16)
        for h_i in range(2):
            xT[:, h_i * B_CORE:(h_i + 1) * B_CORE] = \
                xTc[h_i * 128:(h_i + 1) * 128, :]

        # biasesT[p, h_o*B_CORE + b] = bias_mu[o] + bstd[o]*eps_b[b, o],
        # o = h_o*128 + p
        bT = bias_mu[:, None] + bstd[:, None] * eps_b[sl].T  # (OUT, B_CORE)
        biasesT = np.zeros((128, 2 * B_CORE), np.float32)
        for h_o in range(2):
            biasesT[:, h_o * B_CORE:(h_o + 1) * B_CORE] = \
                bT[h_o * 128:(h_o + 1) * 128, :]

        in_maps.append({
            "eps": eps_w[sl],
            "srepT": srepT,
            "srepOI": srepOI,
            "xT": xT,
            "muT": muT,
            "biasesT": biasesT,
            "ident": ident,
        })
    return in_maps


def run(trace=False, **inputs):
    nc = _get_nc()
    in_maps = _prep_inputs(**inputs)
    res = run_bass_kernel_spmd(nc, in_maps, list(range(N_CORES)), trace=trace)
    out = np.concatenate(
        [np.asarray(res.results[c]["out"]).T for c in range(N_CORES)], axis=0)
    return np.ascontiguousarray(out, np.float32), res


def kernel(**inputs) -> np.ndarray:
    out, _ = run(trace=False, **inputs)
    return out
